# revision 11
# baseline (speedup 1.0000x reference)
"""ComplEx + KBLN scoring kernel for 8 Trainium2 NeuronCores.

Math:
  score_l[b,e] = u[b] @ E_real[e] + v[b] @ E_img[e]
      u = e1_real*r_real - e1_img*r_img,  v = e1_real*r_img + e1_img*r_real
  phi[b,e,l]  = exp(-((n_h[b,l] - lit[e,l] - c[l])^2) / var[l])
  score_n[b,e] = sum_l w_nf[b,l] * phi[b,e,l]
  out = sigmoid(score_l + score_n)

Device algorithm (per core, entities sharded 8 ways, no collectives):
  With a'[b,l] = (n_h[b,l]-c[l])*s[l], t[l,e] = lit[e,l]*s[l], s = 1/sqrt(var):
      phi = exp(-(a'-t)^2)
  phi is interpolated in a' over M_NODES Chebyshev nodes x_j spanning the
  (data-dependent) range of a':
      phi(a',t) ~= sum_j L_j(a') * exp(-(x_j-t)^2)
  The node Gaussians are computed once per core via
      exp(-(x_j-t)^2) = exp(-t^2) * exp(2*x_j*t - x_j^2)
  (one ACT Exp pass over the shared T'' tile with scalar-per-partition
  scale/bias supplied as input columns, times a precomputed G = exp(-t^2)),
  and the whole [B,NL] reduction collapses into one fp16 matmul per node:
      score_n[:, e] = sum_j C_j @ P_j[:, e],   C_j[l,b] = w[b,l]*L_j(a'[b,l])
  which accumulates in PSUM on top of score_l's matmul. Interpolation error
  is ~3e-6 at 16 nodes; fp16 operand rounding dominates (~1e-3 on score).

The host side only does O(B*(D+NL)*M_NODES) index gathers and small
transposes; all O(NE) work runs on device.
"""

import numpy as np

import concourse.bass as bass
import concourse.tile as tile
from concourse import bacc, mybir
from concourse.bass_utils import run_bass_kernel_spmd
from concourse.masks import make_identity

B = 128
NE = 14951
D = 200
D2 = 100
NL = 116
NCORES = 8
NE_CORE = 1869          # real entities per core (core 7 has 1868)
NE_PAD = 1920           # padded per-core width: 15 tiles of 128
NCHUNK = 4
CHUNK = NE_PAD // NCHUNK  # 480
MN = 16                 # Chebyshev nodes for the RBF interpolation
F32 = mybir.dt.float32
FP16 = mybir.dt.float16
AF = mybir.ActivationFunctionType


def build_nc():
    nc = bacc.Bacc("TRN2", num_devices=NCORES)

    e_d = nc.dram_tensor("e_slice", [NE_PAD, D], F32, kind="ExternalInput").ap()
    lit_d = nc.dram_tensor("lit_slice", [NE_PAD, NL], F32, kind="ExternalInput").ap()
    nsc_d = nc.dram_tensor("node_scale", [NL, MN], F32, kind="ExternalInput").ap()
    nbi_d = nc.dram_tensor("node_bias", [NL, MN], F32, kind="ExternalInput").ap()
    cmat_d = nc.dram_tensor("cmat", [NL, MN * B], FP16, kind="ExternalInput").ap()
    wu_d = nc.dram_tensor("wu_t", [D2, B], F32, kind="ExternalInput").ap()
    wv_d = nc.dram_tensor("wv_t", [D2, B], F32, kind="ExternalInput").ap()
    s_d = nc.dram_tensor("s_t", [NL, 1], F32, kind="ExternalInput").ap()
    out_d = nc.dram_tensor("out", [B, NE_PAD], F32, kind="ExternalOutput").ap()

    with tile.TileContext(nc) as tc:
        from contextlib import ExitStack

        with ExitStack() as ctx:
            cpool = ctx.enter_context(tc.tile_pool(name="consts", bufs=1))

            node_scale = cpool.tile([NL, MN], F32, name="nsc_sb")
            nc.sync.dma_start(node_scale[:], nsc_d[:])
            node_bias = cpool.tile([NL, MN], F32, name="nbi_sb")
            nc.sync.dma_start(node_bias[:], nbi_d[:])
            cmat = cpool.tile([NL, MN * B], FP16, name="cmat_sb")
            nc.sync.dma_start(cmat[:], cmat_d[:])
            wu_t = cpool.tile([D2, B], F32, name="wu_sb")
            nc.sync.dma_start(wu_t[:], wu_d[:])
            wv_t = cpool.tile([D2, B], F32, name="wv_sb")
            nc.sync.dma_start(wv_t[:], wv_d[:])
            s_t = cpool.tile([NL, 1], F32, name="s_sb")
            nc.sync.dma_start(s_t[:], s_d[:])

            ident = cpool.tile([128, 128], F32, name="ident")
            make_identity(nc, ident[:])

            t2 = cpool.tile([NL, NE_PAD], F32, name="t2")        # lit^T * s
            g = cpool.tile([NL, NE_PAD], F32, name="g")          # exp(-t2^2)
            et_re = cpool.tile([D2, NE_PAD], F32, name="et_re")  # E_real^T
            et_im = cpool.tile([D2, NE_PAD], F32, name="et_im")  # E_img^T

            tpool = ctx.enter_context(tc.tile_pool(name="loadt", bufs=3))
            pspool = ctx.enter_context(tc.tile_pool(name="tpsum", bufs=2, space="PSUM"))

            for i in range(NE_PAD // 128):
                sl = slice(i * 128, (i + 1) * 128)
                lt = tpool.tile([128, NL], F32, name=f"lt{i}", tag="lt")
                nc.sync.dma_start(lt[:], lit_d[sl, :])
                ptl = pspool.tile([NL, 128], F32, name=f"ptl{i}", tag="pt")
                nc.tensor.transpose(ptl[:], lt[:], ident[:])
                nc.vector.tensor_scalar_mul(t2[:, sl], ptl[:], s_t[:])

                et = tpool.tile([128, D], F32, name=f"et{i}", tag="et")
                nc.sync.dma_start(et[:], e_d[sl, :])
                ptr = pspool.tile([D2, 128], F32, name=f"ptr{i}", tag="pt")
                nc.tensor.transpose(ptr[:], et[:, 0:D2], ident[:])
                nc.vector.tensor_copy(et_re[:, sl], ptr[:])
                pti = pspool.tile([D2, 128], F32, name=f"pti{i}", tag="pt")
                nc.tensor.transpose(pti[:], et[:, D2:D], ident[:])
                nc.vector.tensor_copy(et_im[:, sl], pti[:])

            # G = exp(-T''^2), shared by every node pass.
            t2sq = cpool.tile([NL, NE_PAD], F32, name="t2sq")
            nc.scalar.activation(t2sq[:], t2[:], AF.Square)
            nc.scalar.activation(g[:], t2sq[:], AF.Exp, scale=-1.0)

            apool = ctx.enter_context(tc.tile_pool(name="accs", bufs=1, space="PSUM"))
            acc = [
                apool.tile([B, CHUNK], F32, name=f"acc{c}", tag=f"acc{c}")
                for c in range(NCHUNK)
            ]
            for c in range(NCHUNK):
                cs = slice(c * CHUNK, (c + 1) * CHUNK)
                nc.tensor.matmul(acc[c][:, :], wu_t[:], et_re[:, cs], start=True, stop=False)
                nc.tensor.matmul(acc[c][:, :], wv_t[:], et_im[:, cs], start=False, stop=False)

            xpool = ctx.enter_context(tc.tile_pool(name="xs", bufs=3))
            ppool = ctx.enter_context(tc.tile_pool(name="ps", bufs=3))
            for j in range(MN):
                xj = xpool.tile([NL, NE_PAD], F32, name=f"x{j}", tag="xj")
                nc.scalar.activation(
                    xj[:], t2[:], AF.Exp,
                    bias=node_bias[:, j:j + 1], scale=node_scale[:, j:j + 1],
                )
                pj = ppool.tile([NL, NE_PAD], FP16, name=f"p{j}", tag="pj")
                nc.vector.tensor_tensor(pj[:], xj[:], g[:], mybir.AluOpType.mult)
                for c in range(NCHUNK):
                    cs = slice(c * CHUNK, (c + 1) * CHUNK)
                    nc.tensor.matmul(
                        acc[c][:, :],
                        cmat[:, j * B:(j + 1) * B],
                        pj[:, cs],
                        start=False,
                        stop=(j == MN - 1),
                    )

            opool = ctx.enter_context(tc.tile_pool(name="outs", bufs=2))
            for c in range(NCHUNK):
                cs = slice(c * CHUNK, (c + 1) * CHUNK)
                ot = opool.tile([B, CHUNK], F32, name=f"ot{c}", tag="ot")
                nc.scalar.activation(ot[:], acc[c][:, :], AF.Sigmoid)
                nc.sync.dma_start(out_d[:, cs], ot[:])

    nc.compile()
    return nc


_NC_CACHE = None


def _get_nc():
    global _NC_CACHE
    if _NC_CACHE is None:
        _NC_CACHE = build_nc()
    return _NC_CACHE


def _cheb_nodes(lo, hi, m):
    k = np.arange(m)
    x = np.cos((2 * k + 1) * np.pi / (2 * m))
    return (lo + hi) / 2 + (hi - lo) / 2 * x


def host_prep(e1_idx, r_idx, E, R, nf_weights, numerical_literals, c, var):
    """Tiny O(B*(D+NL)*MN) index gathers / small transposes shared by cores."""
    e1_idx = np.asarray(e1_idx).astype(np.int64)
    r_idx = np.asarray(r_idx).astype(np.int64)
    E = np.asarray(E, dtype=np.float32)
    R = np.asarray(R, dtype=np.float32)
    nf_weights = np.asarray(nf_weights, dtype=np.float32)
    numerical_literals = np.asarray(numerical_literals, dtype=np.float32)
    c = np.asarray(c, dtype=np.float32)
    var = np.asarray(var, dtype=np.float32)

    e1 = E[e1_idx]
    r = R[r_idx]
    e1r, e1i = e1[:, :D2], e1[:, D2:]
    rr, ri = r[:, :D2], r[:, D2:]
    u = e1r * rr - e1i * ri
    v = e1r * ri + e1i * rr

    s = (1.0 / np.sqrt(var.astype(np.float64))).astype(np.float64)
    n_h = numerical_literals[e1_idx].astype(np.float64)
    a = (n_h - c[None, :]) * s[None, :]                   # [B, NL]
    w = nf_weights[r_idx].astype(np.float64)              # [B, NL]

    lo, hi = a.min(), a.max()
    half = max((hi - lo) / 2, 1e-6)
    nodes = _cheb_nodes(lo - 1e-9, hi + 1e-9, MN)          # [MN]
    # barycentric Lagrange basis L_j(a[b,l])
    bw = np.ones(MN)
    for j in range(MN):
        bw[j] = 1.0 / np.prod((nodes[j] - np.delete(nodes, j)) / half)
    diff = a[:, :, None] - nodes[None, None, :]            # [B, NL, MN]
    # exact-hit guard (a == node)
    tiny = np.abs(diff) < 1e-12
    diff = np.where(tiny, 1.0, diff)
    tmp = bw[None, None, :] / diff
    tmp = np.where(tiny, 1e18, tmp)
    L = tmp / tmp.sum(-1, keepdims=True)                   # [B, NL, MN]

    C = w[:, :, None] * L                                  # [B, NL, MN]
    cmat = np.ascontiguousarray(
        C.transpose(1, 2, 0).reshape(NL, MN * B)
    ).astype(np.float16)

    node_scale = np.broadcast_to((2.0 * nodes)[None, :], (NL, MN))
    node_bias = np.broadcast_to((-(nodes ** 2))[None, :], (NL, MN))
    return {
        "node_scale": np.ascontiguousarray(node_scale, dtype=np.float32),
        "node_bias": np.ascontiguousarray(node_bias, dtype=np.float32),
        "cmat": cmat,
        "wu_t": np.ascontiguousarray(u.T),
        "wv_t": np.ascontiguousarray(v.T),
        "s_t": s.astype(np.float32).reshape(NL, 1),
    }


def shard_entities(E, numerical_literals):
    """Per-core [NE_PAD, D]/[NE_PAD, NL] slices, zero-padded."""
    E = np.asarray(E, dtype=np.float32)
    lit = np.asarray(numerical_literals, dtype=np.float32)
    e_slices, lit_slices, spans = [], [], []
    for core in range(NCORES):
        lo = core * NE_CORE
        hi = min(NE, lo + NE_CORE)
        es = np.zeros((NE_PAD, D), dtype=np.float32)
        ls = np.zeros((NE_PAD, NL), dtype=np.float32)
        es[: hi - lo] = E[lo:hi]
        ls[: hi - lo] = lit[lo:hi]
        e_slices.append(es)
        lit_slices.append(ls)
        spans.append((lo, hi))
    return e_slices, lit_slices, spans


def _make_in_maps(inputs):
    small = host_prep(**inputs)
    e_slices, lit_slices, spans = shard_entities(
        inputs["E"], inputs["numerical_literals"]
    )
    in_maps = []
    for core in range(NCORES):
        m = dict(small)
        m["e_slice"] = e_slices[core]
        m["lit_slice"] = lit_slices[core]
        in_maps.append(m)
    return in_maps, spans


def run_on_device(inputs, trace=False):
    nc = _get_nc()
    in_maps, spans = _make_in_maps(inputs)
    res = run_bass_kernel_spmd(nc, in_maps, core_ids=list(range(NCORES)), trace=trace)
    out = np.empty((B, NE), dtype=np.float32)
    for core, (lo, hi) in enumerate(spans):
        out[:, lo:hi] = res.results[core]["out"][:, : hi - lo]
    return out, res


def kernel(**inputs):
    out, _ = run_on_device(inputs, trace=False)
    return out


def bench(inputs, iters=8, reps=5):
    """Estimate per-NEFF-execution device time by chaining `iters` executions
    of the kernel inside one jitted program (each feeds its output buffer to
    the next, forcing sequential execution) and differencing against a
    single-execution program."""
    import jax
    from jax.sharding import Mesh, PartitionSpec
    try:
        from jax.experimental.shard_map import shard_map
    except ImportError:
        from jax.shard_map import shard_map
    from concourse import bass2jax
    import time

    nc = _get_nc()
    bass2jax.install_neuronx_cc_hook()
    in_maps, _ = _make_in_maps(inputs)

    partition_name = nc.partition_id_tensor.name if nc.partition_id_tensor else None
    in_names, out_names, out_avals, zero_outs = [], [], [], []
    for alloc in nc.m.functions[0].allocations:
        if not isinstance(alloc, mybir.MemoryLocationSet):
            continue
        name = alloc.memorylocations[0].name
        if alloc.kind == "ExternalInput":
            if name != partition_name:
                in_names.append(name)
        elif alloc.kind == "ExternalOutput":
            shape = tuple(alloc.tensor_shape)
            dtype = mybir.dt.np(alloc.dtype)
            out_avals.append(jax.core.ShapedArray(shape, dtype))
            out_names.append(name)
            zero_outs.append(np.zeros(shape, dtype))
    n_params = len(in_names)
    all_names = list(in_names) + list(out_names)
    if partition_name is not None:
        all_names.append(partition_name)

    def make_body(k):
        def _body(*args):
            ins = list(args[:n_params])
            bufs = list(args[n_params:])
            for _ in range(k):
                operands = ins + bufs
                if partition_name is not None:
                    operands.append(bass2jax.partition_id_tensor())
                bufs = list(bass2jax._bass_exec_p.bind(
                    *operands,
                    out_avals=tuple(out_avals),
                    in_names=tuple(all_names),
                    out_names=tuple(out_names),
                    lowering_input_output_aliases=(),
                    sim_require_finite=True,
                    sim_require_nnan=True,
                    nc=nc,
                ))
            return tuple(bufs)
        return _body

    devices = jax.devices()[:NCORES]
    mesh = Mesh(np.asarray(devices), ("core",))
    nin = n_params + len(out_avals)
    per_core = [[np.asarray(m[nm]) for nm in in_names] for m in in_maps]
    concat_in = [np.concatenate([per_core[c][i] for c in range(NCORES)], axis=0)
                 for i in range(n_params)]
    concat_zeros = [np.zeros((NCORES * z.shape[0], *z.shape[1:]), z.dtype)
                    for z in zero_outs]

    f1 = jax.jit(shard_map(
        make_body(1), mesh=mesh,
        in_specs=(PartitionSpec("core"),) * nin,
        out_specs=(PartitionSpec("core"),) * len(out_names),
        check_rep=False))

    args_dev = jax.device_put(
        concat_in + concat_zeros,
        [jax.sharding.NamedSharding(mesh, PartitionSpec("core"))] * nin)

    # dispatch-overhead probe: trivial sharded computation on the same mesh
    probe_in = args_dev[n_params]  # an output-shaped buffer
    f0 = jax.jit(shard_map(
        lambda x: x + 1.0, mesh=mesh,
        in_specs=PartitionSpec("core"), out_specs=PartitionSpec("core"),
        check_rep=False))

    def timeit(f, args, n):
        jax.block_until_ready(f(*args))  # compile + warm
        best = float("inf")
        for _ in range(n):
            t0 = time.perf_counter()
            jax.block_until_ready(f(*args))
            best = min(best, time.perf_counter() - t0)
        return best

    t_probe = timeit(f0, (probe_in,), max(reps, 20))
    t_kern = timeit(f1, args_dev, max(reps, 20))
    per_iter = t_kern - t_probe
    print(f"bench: kernel call {t_kern*1e6:.1f} us  dispatch probe {t_probe*1e6:.1f} us"
          f"  -> per-exec ~{per_iter*1e6:.1f} us")
    return per_iter * 1e9


# revision 14
# speedup vs baseline: 42.9774x; 42.9774x over previous
"""ComplEx + KBLN scoring kernel for 8 Trainium2 NeuronCores.

Math:
  score_l[b,e] = u[b] @ E_real[e] + v[b] @ E_img[e]
      u = e1_real*r_real - e1_img*r_img,  v = e1_real*r_img + e1_img*r_real
  phi[b,e,l]  = exp(-((n_h[b,l] - lit[e,l] - c[l])^2) / var[l])
  score_n[b,e] = sum_l w_nf[b,l] * phi[b,e,l]
  out = sigmoid(score_l + score_n)

Device algorithm (per core, entities sharded 8 ways, no collectives):
  With a'[b,l] = (n_h[b,l]-c[l])*s[l], t[l,e] = lit[e,l]*s[l], s = 1/sqrt(var):
      phi = exp(-(a'-t)^2)
  phi is interpolated in a' over M_NODES Chebyshev nodes x_j spanning the
  (data-dependent) range of a':
      phi(a',t) ~= sum_j L_j(a') * exp(-(x_j-t)^2)
  The node Gaussians are computed once per core via
      exp(-(x_j-t)^2) = exp(-t^2) * exp(2*x_j*t - x_j^2)
  (one ACT Exp pass over the shared T'' tile with scalar-per-partition
  scale/bias supplied as input columns, times a precomputed G = exp(-t^2)),
  and the whole [B,NL] reduction collapses into one fp16 matmul per node:
      score_n[:, e] = sum_j C_j @ P_j[:, e],   C_j[l,b] = w[b,l]*L_j(a'[b,l])
  which accumulates in PSUM on top of score_l's matmul. Interpolation error
  is ~3e-6 at 16 nodes; fp16 operand rounding dominates (~1e-3 on score).

The host side only does O(B*(D+NL)*M_NODES) index gathers and small
transposes; all O(NE) work runs on device.
"""

import numpy as np

import concourse.bass as bass
import concourse.tile as tile
from concourse import bacc, mybir
from concourse.bass_utils import run_bass_kernel_spmd
from concourse.masks import make_identity

B = 128
NE = 14951
D = 200
D2 = 100
NL = 116
NCORES = 8
NE_CORE = 1869          # real entities per core (core 7 has 1868)
NE_PAD = 1920           # padded per-core width: 15 tiles of 128
NCHUNK = 4
CHUNK = NE_PAD // NCHUNK  # 480
MN = 12                 # Chebyshev nodes for the RBF interpolation
F32 = mybir.dt.float32
FP16 = mybir.dt.float16
AF = mybir.ActivationFunctionType


def _emit_body(nc, tc, ctx, pools, aps, r):
    """One full evaluation of the kernel. `r` prefixes tile names so the body
    can be instantiated multiple times (benchmark builds)."""
    e_d, lit_d, nsc_d, nbi_d, cmat_d, wu_d, wv_d, s_d, out_d = aps
    cpool, tpool, pspool, apool, xpool, ppool, opool = pools

    node_scale = cpool.tile([NL, MN], F32, name=f"{r}nsc_sb", tag="nsc")
    nc.sync.dma_start(node_scale[:], nsc_d[:])
    node_bias = cpool.tile([NL, MN], F32, name=f"{r}nbi_sb", tag="nbi")
    nc.sync.dma_start(node_bias[:], nbi_d[:])
    cmat = cpool.tile([NL, MN * B], FP16, name=f"{r}cmat_sb", tag="cmat")
    nc.sync.dma_start(cmat[:], cmat_d[:])
    wu_t = cpool.tile([D2, B], F32, name=f"{r}wu_sb", tag="wu")
    nc.sync.dma_start(wu_t[:], wu_d[:])
    wv_t = cpool.tile([D2, B], F32, name=f"{r}wv_sb", tag="wv")
    nc.sync.dma_start(wv_t[:], wv_d[:])
    s_t = cpool.tile([NL, 1], F32, name=f"{r}s_sb", tag="st")
    nc.sync.dma_start(s_t[:], s_d[:])

    ident = cpool.tile([128, 128], F32, name=f"{r}ident", tag="ident")
    make_identity(nc, ident[:])

    t2 = cpool.tile([NL, NE_PAD], F32, name=f"{r}t2", tag="t2")
    g = cpool.tile([NL, NE_PAD], F32, name=f"{r}g", tag="g")
    et_re = cpool.tile([D2, NE_PAD], F32, name=f"{r}et_re", tag="et_re")
    et_im = cpool.tile([D2, NE_PAD], F32, name=f"{r}et_im", tag="et_im")

    for i in range(NE_PAD // 128):
        sl = slice(i * 128, (i + 1) * 128)
        lt = tpool.tile([128, NL], F32, name=f"{r}lt{i}", tag="lt")
        nc.sync.dma_start(lt[:], lit_d[sl, :])
        ptl = pspool.tile([NL, 128], F32, name=f"{r}ptl{i}", tag="pt")
        nc.tensor.transpose(ptl[:], lt[:], ident[:])
        nc.vector.tensor_scalar_mul(t2[:, sl], ptl[:], s_t[:])

        et = tpool.tile([128, D], F32, name=f"{r}et{i}", tag="et")
        nc.sync.dma_start(et[:], e_d[sl, :])
        ptr = pspool.tile([D2, 128], F32, name=f"{r}ptr{i}", tag="pt")
        nc.tensor.transpose(ptr[:], et[:, 0:D2], ident[:])
        nc.vector.tensor_copy(et_re[:, sl], ptr[:])
        pti = pspool.tile([D2, 128], F32, name=f"{r}pti{i}", tag="pt")
        nc.tensor.transpose(pti[:], et[:, D2:D], ident[:])
        nc.vector.tensor_copy(et_im[:, sl], pti[:])

    # G = exp(-T''^2), shared by every node pass.
    t2sq = cpool.tile([NL, NE_PAD], F32, name=f"{r}t2sq", tag="t2sq")
    nc.scalar.activation(t2sq[:], t2[:], AF.Square)
    nc.scalar.activation(g[:], t2sq[:], AF.Exp, scale=-1.0)

    acc = [
        apool.tile([B, CHUNK], F32, name=f"{r}acc{c}", tag=f"acc{c}")
        for c in range(NCHUNK)
    ]
    for j in range(MN):
        xj = xpool.tile([NL, NE_PAD], F32, name=f"{r}x{j}", tag="xj")
        nc.scalar.activation(
            xj[:], t2[:], AF.Exp,
            bias=node_bias[:, j:j + 1], scale=node_scale[:, j:j + 1],
        )
        pj = ppool.tile([NL, NE_PAD], FP16, name=f"{r}p{j}", tag="pj")
        nc.vector.tensor_tensor(pj[:], xj[:], g[:], mybir.AluOpType.mult)
        for c in range(NCHUNK):
            cs = slice(c * CHUNK, (c + 1) * CHUNK)
            nc.tensor.matmul(
                acc[c][:, :],
                cmat[:, j * B:(j + 1) * B],
                pj[:, cs],
                start=(j == 0),
                stop=False,
            )

    # score_l accumulates last so the E DMA + transposes overlap the node
    # pipeline above.
    for c in range(NCHUNK):
        cs = slice(c * CHUNK, (c + 1) * CHUNK)
        nc.tensor.matmul(acc[c][:, :], wu_t[:], et_re[:, cs], start=False, stop=False)
        nc.tensor.matmul(acc[c][:, :], wv_t[:], et_im[:, cs], start=False, stop=True)

    for c in range(NCHUNK):
        cs = slice(c * CHUNK, (c + 1) * CHUNK)
        ot = opool.tile([B, CHUNK], F32, name=f"{r}ot{c}", tag="ot")
        nc.scalar.activation(ot[:], acc[c][:, :], AF.Sigmoid)
        nc.sync.dma_start(out_d[:, cs], ot[:])


def build_nc(reps=1):
    nc = bacc.Bacc("TRN2", num_devices=NCORES)

    aps = (
        nc.dram_tensor("e_slice", [NE_PAD, D], F32, kind="ExternalInput").ap(),
        nc.dram_tensor("lit_slice", [NE_PAD, NL], F32, kind="ExternalInput").ap(),
        nc.dram_tensor("node_scale", [NL, MN], F32, kind="ExternalInput").ap(),
        nc.dram_tensor("node_bias", [NL, MN], F32, kind="ExternalInput").ap(),
        nc.dram_tensor("cmat", [NL, MN * B], FP16, kind="ExternalInput").ap(),
        nc.dram_tensor("wu_t", [D2, B], F32, kind="ExternalInput").ap(),
        nc.dram_tensor("wv_t", [D2, B], F32, kind="ExternalInput").ap(),
        nc.dram_tensor("s_t", [NL, 1], F32, kind="ExternalInput").ap(),
        nc.dram_tensor("out", [B, NE_PAD], F32, kind="ExternalOutput").ap(),
    )

    with tile.TileContext(nc) as tc:
        from contextlib import ExitStack

        with ExitStack() as ctx:
            pools = (
                ctx.enter_context(tc.tile_pool(name="consts", bufs=2)),
                ctx.enter_context(tc.tile_pool(name="loadt", bufs=3)),
                ctx.enter_context(tc.tile_pool(name="tpsum", bufs=2, space="PSUM")),
                ctx.enter_context(tc.tile_pool(name="accs", bufs=1, space="PSUM")),
                ctx.enter_context(tc.tile_pool(name="xs", bufs=3)),
                ctx.enter_context(tc.tile_pool(name="ps", bufs=3)),
                ctx.enter_context(tc.tile_pool(name="outs", bufs=2)),
            )
            for rep in range(reps):
                _emit_body(nc, tc, ctx, pools, aps, f"r{rep}_" if reps > 1 else "")

    nc.compile()
    return nc


_NC_CACHE = {}


def _get_nc(reps=1):
    if reps not in _NC_CACHE:
        _NC_CACHE[reps] = build_nc(reps)
    return _NC_CACHE[reps]


def _cheb_nodes(lo, hi, m):
    k = np.arange(m)
    x = np.cos((2 * k + 1) * np.pi / (2 * m))
    return (lo + hi) / 2 + (hi - lo) / 2 * x


def host_prep(e1_idx, r_idx, E, R, nf_weights, numerical_literals, c, var):
    """Tiny O(B*(D+NL)*MN) index gathers / small transposes shared by cores."""
    e1_idx = np.asarray(e1_idx).astype(np.int64)
    r_idx = np.asarray(r_idx).astype(np.int64)
    E = np.asarray(E, dtype=np.float32)
    R = np.asarray(R, dtype=np.float32)
    nf_weights = np.asarray(nf_weights, dtype=np.float32)
    numerical_literals = np.asarray(numerical_literals, dtype=np.float32)
    c = np.asarray(c, dtype=np.float32)
    var = np.asarray(var, dtype=np.float32)

    e1 = E[e1_idx]
    r = R[r_idx]
    e1r, e1i = e1[:, :D2], e1[:, D2:]
    rr, ri = r[:, :D2], r[:, D2:]
    u = e1r * rr - e1i * ri
    v = e1r * ri + e1i * rr

    s = (1.0 / np.sqrt(var.astype(np.float64))).astype(np.float64)
    n_h = numerical_literals[e1_idx].astype(np.float64)
    a = (n_h - c[None, :]) * s[None, :]                   # [B, NL]
    w = nf_weights[r_idx].astype(np.float64)              # [B, NL]

    lo, hi = a.min(), a.max()
    half = max((hi - lo) / 2, 1e-6)
    nodes = _cheb_nodes(lo - 1e-9, hi + 1e-9, MN)          # [MN]
    # barycentric Lagrange basis L_j(a[b,l])
    bw = np.ones(MN)
    for j in range(MN):
        bw[j] = 1.0 / np.prod((nodes[j] - np.delete(nodes, j)) / half)
    diff = a[:, :, None] - nodes[None, None, :]            # [B, NL, MN]
    # exact-hit guard (a == node)
    tiny = np.abs(diff) < 1e-12
    diff = np.where(tiny, 1.0, diff)
    tmp = bw[None, None, :] / diff
    tmp = np.where(tiny, 1e18, tmp)
    L = tmp / tmp.sum(-1, keepdims=True)                   # [B, NL, MN]

    C = w[:, :, None] * L                                  # [B, NL, MN]
    cmat = np.ascontiguousarray(
        C.transpose(1, 2, 0).reshape(NL, MN * B)
    ).astype(np.float16)

    node_scale = np.broadcast_to((2.0 * nodes)[None, :], (NL, MN))
    node_bias = np.broadcast_to((-(nodes ** 2))[None, :], (NL, MN))
    return {
        "node_scale": np.ascontiguousarray(node_scale, dtype=np.float32),
        "node_bias": np.ascontiguousarray(node_bias, dtype=np.float32),
        "cmat": cmat,
        "wu_t": np.ascontiguousarray(u.T),
        "wv_t": np.ascontiguousarray(v.T),
        "s_t": s.astype(np.float32).reshape(NL, 1),
    }


def shard_entities(E, numerical_literals):
    """Per-core [NE_PAD, D]/[NE_PAD, NL] slices, zero-padded."""
    E = np.asarray(E, dtype=np.float32)
    lit = np.asarray(numerical_literals, dtype=np.float32)
    e_slices, lit_slices, spans = [], [], []
    for core in range(NCORES):
        lo = core * NE_CORE
        hi = min(NE, lo + NE_CORE)
        es = np.zeros((NE_PAD, D), dtype=np.float32)
        ls = np.zeros((NE_PAD, NL), dtype=np.float32)
        es[: hi - lo] = E[lo:hi]
        ls[: hi - lo] = lit[lo:hi]
        e_slices.append(es)
        lit_slices.append(ls)
        spans.append((lo, hi))
    return e_slices, lit_slices, spans


def _make_in_maps(inputs):
    small = host_prep(**inputs)
    e_slices, lit_slices, spans = shard_entities(
        inputs["E"], inputs["numerical_literals"]
    )
    in_maps = []
    for core in range(NCORES):
        m = dict(small)
        m["e_slice"] = e_slices[core]
        m["lit_slice"] = lit_slices[core]
        in_maps.append(m)
    return in_maps, spans


def run_on_device(inputs, trace=False):
    nc = _get_nc()
    in_maps, spans = _make_in_maps(inputs)
    res = run_bass_kernel_spmd(nc, in_maps, core_ids=list(range(NCORES)), trace=trace)
    out = np.empty((B, NE), dtype=np.float32)
    for core, (lo, hi) in enumerate(spans):
        out[:, lo:hi] = res.results[core]["out"][:, : hi - lo]
    return out, res


def kernel(**inputs):
    out, _ = run_on_device(inputs, trace=False)
    return out


def _make_runner(nc, in_maps):
    """Build a reusable jitted callable + device-resident args for `nc`."""
    import jax
    from jax.sharding import Mesh, PartitionSpec
    try:
        from jax.experimental.shard_map import shard_map
    except ImportError:
        from jax.shard_map import shard_map
    from concourse import bass2jax

    bass2jax.install_neuronx_cc_hook()
    partition_name = nc.partition_id_tensor.name if nc.partition_id_tensor else None
    in_names, out_names, out_avals, zero_outs = [], [], [], []
    for alloc in nc.m.functions[0].allocations:
        if not isinstance(alloc, mybir.MemoryLocationSet):
            continue
        name = alloc.memorylocations[0].name
        if alloc.kind == "ExternalInput":
            if name != partition_name:
                in_names.append(name)
        elif alloc.kind == "ExternalOutput":
            shape = tuple(alloc.tensor_shape)
            dtype = mybir.dt.np(alloc.dtype)
            out_avals.append(jax.core.ShapedArray(shape, dtype))
            out_names.append(name)
            zero_outs.append(np.zeros(shape, dtype))
    n_params = len(in_names)
    all_names = list(in_names) + list(out_names)
    if partition_name is not None:
        all_names.append(partition_name)

    def _body(*args):
        operands = list(args)
        if partition_name is not None:
            operands.append(bass2jax.partition_id_tensor())
        return tuple(bass2jax._bass_exec_p.bind(
            *operands,
            out_avals=tuple(out_avals),
            in_names=tuple(all_names),
            out_names=tuple(out_names),
            lowering_input_output_aliases=(),
            sim_require_finite=True,
            sim_require_nnan=True,
            nc=nc,
        ))

    devices = jax.devices()[:NCORES]
    mesh = Mesh(np.asarray(devices), ("core",))
    nin = n_params + len(out_avals)
    per_core = [[np.asarray(m[nm]) for nm in in_names] for m in in_maps]
    concat_in = [np.concatenate([per_core[c][i] for c in range(NCORES)], axis=0)
                 for i in range(n_params)]
    concat_zeros = [np.zeros((NCORES * z.shape[0], *z.shape[1:]), z.dtype)
                    for z in zero_outs]
    f = jax.jit(shard_map(
        _body, mesh=mesh,
        in_specs=(PartitionSpec("core"),) * nin,
        out_specs=(PartitionSpec("core"),) * len(out_names),
        check_rep=False))
    args_dev = jax.device_put(
        concat_in + concat_zeros,
        [jax.sharding.NamedSharding(mesh, PartitionSpec("core"))] * nin)
    return f, args_dev


def bench(inputs, reps_program=64, timing_reps=30):
    """Per-execution device time: difference a program with the kernel body
    instantiated `reps_program` times against the 1-rep program. The (large,
    ~90 ms) axon dispatch overhead cancels in the difference."""
    import jax
    import time

    in_maps, _ = _make_in_maps(inputs)

    def timeit(f, args, n):
        jax.block_until_ready(f(*args))
        best = float("inf")
        for _ in range(n):
            t0 = time.perf_counter()
            jax.block_until_ready(f(*args))
            best = min(best, time.perf_counter() - t0)
        return best

    f1, a1 = _make_runner(_get_nc(1), in_maps)
    fR, aR = _make_runner(_get_nc(reps_program), in_maps)
    # warm both (compile + first dispatch)
    jax.block_until_ready(f1(*a1))
    jax.block_until_ready(fR(*aR))
    # interleave to cancel axon dispatch-time drift
    diffs = []
    for _ in range(timing_reps):
        t0 = time.perf_counter()
        jax.block_until_ready(f1(*a1))
        t1 = time.perf_counter()
        jax.block_until_ready(fR(*aR))
        t2 = time.perf_counter()
        diffs.append((t2 - t1) - (t1 - t0))
    diffs.sort()
    med = diffs[len(diffs) // 2]
    per = med / (reps_program - 1)
    print(f"bench: median extra for {reps_program - 1} reps = {med*1e3:.3f} ms"
          f"  -> per-exec {per*1e6:.1f} us"
          f"  (p25 {diffs[len(diffs)//4]/(reps_program-1)*1e6:.1f},"
          f" p75 {diffs[3*len(diffs)//4]/(reps_program-1)*1e6:.1f})")
    return per * 1e9


# revision 24
# speedup vs baseline: 63.4932x; 1.4774x over previous
"""ComplEx + KBLN scoring kernel for 8 Trainium2 NeuronCores.

Math:
  score_l[b,e] = u[b] @ E_real[e] + v[b] @ E_img[e]
      u = e1_real*r_real - e1_img*r_img,  v = e1_real*r_img + e1_img*r_real
  phi[b,e,l]  = exp(-((n_h[b,l] - lit[e,l] - c[l])^2) / var[l])
  score_n[b,e] = sum_l w_nf[b,l] * phi[b,e,l]
  out = sigmoid(score_l + score_n)

Device algorithm (per core, entities sharded 8 ways, no collectives):
  With a'[b,l] = (n_h[b,l]-c[l])*s[l], t[l,e] = lit[e,l]*s[l], s = 1/sqrt(var):
      phi = exp(-(a'-t)^2)
  phi is interpolated in a' over M_NODES Chebyshev nodes x_j spanning the
  (data-dependent) range of a':
      phi(a',t) ~= sum_j L_j(a') * exp(-(x_j-t)^2)
  The node Gaussians are computed once per core via
      exp(-(x_j-t)^2) = exp(-t^2) * exp(2*x_j*t - x_j^2)
  (one ACT Exp pass over the shared T'' tile with scalar-per-partition
  scale/bias supplied as input columns, times a precomputed G = exp(-t^2)),
  and the whole [B,NL] reduction collapses into one fp16 matmul per node:
      score_n[:, e] = sum_j C_j @ P_j[:, e],   C_j[l,b] = w[b,l]*L_j(a'[b,l])
  which accumulates in PSUM on top of score_l's matmul. Interpolation error
  is ~3e-6 at 16 nodes; fp16 operand rounding dominates (~1e-3 on score).

The host side only does O(B*(D+NL)*M_NODES) index gathers and small
transposes; all O(NE) work runs on device.
"""

import ml_dtypes
import numpy as np

import concourse.bass as bass
import concourse.tile as tile
from concourse import bacc, mybir
from concourse.bass_utils import run_bass_kernel_spmd
from concourse.masks import make_identity

B = 128
NE = 14951
D = 200
D2 = 100
NL = 116
NCORES = 8
NE_CORE = 1869          # real entities per core (core 7 has 1868)
NE_PAD = 1920           # padded per-core width: 15 tiles of 128
NCHUNK = 4
CHUNK = NE_PAD // NCHUNK  # 480
MN = 12                 # Chebyshev nodes for the RBF interpolation
F32 = mybir.dt.float32
FP16 = mybir.dt.float16
BF16 = mybir.dt.bfloat16
AF = mybir.ActivationFunctionType


def _emit_body(nc, tc, ctx, pools, aps, r):
    """One full evaluation of the kernel. `r` prefixes tile names so the body
    can be instantiated multiple times (benchmark builds)."""
    e_d, lit_d, nsc_d, nbi_d, cmat_d, wu_d, wv_d, s_d, out_d = aps
    cpool, tpool, pspool, apool, xpool, ppool, opool = pools

    node_scale = cpool.tile([NL, MN], F32, name=f"{r}nsc_sb", tag="nsc")
    nc.sync.dma_start(node_scale[:], nsc_d[:])
    node_bias = cpool.tile([NL, MN], F32, name=f"{r}nbi_sb", tag="nbi")
    nc.sync.dma_start(node_bias[:], nbi_d[:])
    cmat = cpool.tile([NL, MN * B], FP16, name=f"{r}cmat_sb", tag="cmat")
    nc.sync.dma_start(cmat[:], cmat_d[:])
    wu_t = cpool.tile([D2, B], BF16, name=f"{r}wu_sb", tag="wu")
    nc.sync.dma_start(wu_t[:], wu_d[:])
    wv_t = cpool.tile([D2, B], BF16, name=f"{r}wv_sb", tag="wv")
    nc.sync.dma_start(wv_t[:], wv_d[:])
    s_t = cpool.tile([NL, 1], F32, name=f"{r}s_sb", tag="st")
    nc.sync.dma_start(s_t[:], s_d[:])

    ident = cpool.tile([128, 128], F32, name=f"{r}ident", tag="ident")
    make_identity(nc, ident[:])
    identb = cpool.tile([128, 128], BF16, name=f"{r}identb", tag="identb")
    make_identity(nc, identb[:])

    t2 = cpool.tile([NL, NE_PAD], F32, name=f"{r}t2", tag="t2")
    g = cpool.tile([NL, NE_PAD], F32, name=f"{r}g", tag="g")
    et_re = cpool.tile([D2, NE_PAD], BF16, name=f"{r}et_re", tag="et_re")
    et_im = cpool.tile([D2, NE_PAD], BF16, name=f"{r}et_im", tag="et_im")

    t2sq = cpool.tile([NL, NE_PAD], F32, name=f"{r}t2sq", tag="t2sq")

    # lit first: T'' gates the whole node pipeline; E is only needed by the
    # trailing score_l matmuls.
    for i in range(NE_PAD // 128):
        sl = slice(i * 128, (i + 1) * 128)
        lt = tpool.tile([128, NL], F32, name=f"{r}lt{i}", tag="lt")
        nc.sync.dma_start(lt[:], lit_d[sl, :])
        ptl = pspool.tile([NL, 128], F32, name=f"{r}ptl{i}", tag="pt")
        nc.tensor.transpose(ptl[:], lt[:], ident[:])
        nc.vector.tensor_scalar_mul(t2[:, sl], ptl[:], s_t[:])
        nc.vector.tensor_tensor(t2sq[:, sl], t2[:, sl], t2[:, sl],
                                mybir.AluOpType.mult)

    for i in range(NE_PAD // 128):
        sl = slice(i * 128, (i + 1) * 128)
        et = tpool.tile([128, D], BF16, name=f"{r}et{i}", tag="et")
        nc.sync.dma_start(et[:], e_d[sl, :])
        ptr = pspool.tile([D2, 128], BF16, name=f"{r}ptr{i}", tag="ptb")
        nc.tensor.transpose(ptr[:], et[:, 0:D2], identb[:])
        nc.vector.tensor_copy(et_re[:, sl], ptr[:])
        pti = pspool.tile([D2, 128], BF16, name=f"{r}pti{i}", tag="ptb")
        nc.tensor.transpose(pti[:], et[:, D2:D], identb[:])
        nc.vector.tensor_copy(et_im[:, sl], pti[:])

    # G = exp(-T''^2), shared by every node pass.
    nc.scalar.activation(g[:], t2sq[:], AF.Exp, scale=-1.0)

    acc = [
        apool.tile([B, CHUNK], F32, name=f"{r}acc{c}", tag=f"acc{c}")
        for c in range(NCHUNK)
    ]
    for j in range(MN):
        xj = xpool.tile([NL, NE_PAD], F32, name=f"{r}x{j}", tag="xj")
        nc.scalar.activation(
            xj[:], t2[:], AF.Exp,
            bias=node_bias[:, j:j + 1], scale=node_scale[:, j:j + 1],
        )
        pj = ppool.tile([NL, NE_PAD], FP16, name=f"{r}p{j}", tag="pj")
        nc.vector.tensor_tensor(pj[:], xj[:], g[:], mybir.AluOpType.mult)
        for c in range(NCHUNK):
            cs = slice(c * CHUNK, (c + 1) * CHUNK)
            nc.tensor.matmul(
                acc[c][:, :],
                cmat[:, j * B:(j + 1) * B],
                pj[:, cs],
                start=(j == 0),
                stop=False,
            )

    # score_l accumulates last so the E DMA + transposes overlap the node
    # pipeline above.
    for c in range(NCHUNK):
        cs = slice(c * CHUNK, (c + 1) * CHUNK)
        nc.tensor.matmul(acc[c][:, :], wu_t[:], et_re[:, cs], start=False, stop=False)
        nc.tensor.matmul(acc[c][:, :], wv_t[:], et_im[:, cs], start=False, stop=True)

    for c in range(NCHUNK):
        cs = slice(c * CHUNK, (c + 1) * CHUNK)
        ot = opool.tile([B, CHUNK], F32, name=f"{r}ot{c}", tag="ot")
        nc.scalar.activation(ot[:], acc[c][:, :], AF.Sigmoid)
        nc.sync.dma_start(out_d[:, cs], ot[:])


def build_nc(reps=1):
    nc = bacc.Bacc("TRN2", num_devices=NCORES)

    aps = (
        nc.dram_tensor("e_slice", [NE_PAD, D], BF16, kind="ExternalInput").ap(),
        nc.dram_tensor("lit_slice", [NE_PAD, NL], F32, kind="ExternalInput").ap(),
        nc.dram_tensor("node_scale", [NL, MN], F32, kind="ExternalInput").ap(),
        nc.dram_tensor("node_bias", [NL, MN], F32, kind="ExternalInput").ap(),
        nc.dram_tensor("cmat", [NL, MN * B], FP16, kind="ExternalInput").ap(),
        nc.dram_tensor("wu_t", [D2, B], BF16, kind="ExternalInput").ap(),
        nc.dram_tensor("wv_t", [D2, B], BF16, kind="ExternalInput").ap(),
        nc.dram_tensor("s_t", [NL, 1], F32, kind="ExternalInput").ap(),
        nc.dram_tensor("out", [B, NE_PAD], F32, kind="ExternalOutput").ap(),
    )

    with tile.TileContext(nc) as tc:
        from contextlib import ExitStack

        with ExitStack() as ctx:
            pools = (
                ctx.enter_context(tc.tile_pool(name="consts", bufs=2)),
                ctx.enter_context(tc.tile_pool(name="loadt", bufs=3)),
                ctx.enter_context(tc.tile_pool(name="tpsum", bufs=2, space="PSUM")),
                ctx.enter_context(tc.tile_pool(name="accs", bufs=1, space="PSUM")),
                ctx.enter_context(tc.tile_pool(name="xs", bufs=3)),
                ctx.enter_context(tc.tile_pool(name="ps", bufs=3)),
                ctx.enter_context(tc.tile_pool(name="outs", bufs=2)),
            )
            for rep in range(reps):
                _emit_body(nc, tc, ctx, pools, aps, f"r{rep}_" if reps > 1 else "")

    nc.compile()
    return nc


_NC_CACHE = {}


def _get_nc(reps=1):
    if reps not in _NC_CACHE:
        _NC_CACHE[reps] = build_nc(reps)
    return _NC_CACHE[reps]


def _cheb_nodes(lo, hi, m):
    k = np.arange(m)
    x = np.cos((2 * k + 1) * np.pi / (2 * m))
    return (lo + hi) / 2 + (hi - lo) / 2 * x


def host_prep(e1_idx, r_idx, E, R, nf_weights, numerical_literals, c, var):
    """Tiny O(B*(D+NL)*MN) index gathers / small transposes shared by cores."""
    e1_idx = np.asarray(e1_idx).astype(np.int64)
    r_idx = np.asarray(r_idx).astype(np.int64)
    E = np.asarray(E, dtype=np.float32)
    R = np.asarray(R, dtype=np.float32)
    nf_weights = np.asarray(nf_weights, dtype=np.float32)
    numerical_literals = np.asarray(numerical_literals, dtype=np.float32)
    c = np.asarray(c, dtype=np.float32)
    var = np.asarray(var, dtype=np.float32)

    e1 = E[e1_idx]
    r = R[r_idx]
    e1r, e1i = e1[:, :D2], e1[:, D2:]
    rr, ri = r[:, :D2], r[:, D2:]
    u = e1r * rr - e1i * ri
    v = e1r * ri + e1i * rr

    s = (1.0 / np.sqrt(var.astype(np.float64))).astype(np.float64)
    n_h = numerical_literals[e1_idx].astype(np.float64)
    a = (n_h - c[None, :]) * s[None, :]                   # [B, NL]
    w = nf_weights[r_idx].astype(np.float64)              # [B, NL]

    lo, hi = a.min(), a.max()
    half = max((hi - lo) / 2, 1e-6)
    nodes = _cheb_nodes(lo - 1e-9, hi + 1e-9, MN)          # [MN]
    # barycentric Lagrange basis L_j(a[b,l])
    bw = np.ones(MN)
    for j in range(MN):
        bw[j] = 1.0 / np.prod((nodes[j] - np.delete(nodes, j)) / half)
    diff = a[:, :, None] - nodes[None, None, :]            # [B, NL, MN]
    # exact-hit guard (a == node)
    tiny = np.abs(diff) < 1e-12
    diff = np.where(tiny, 1.0, diff)
    tmp = bw[None, None, :] / diff
    tmp = np.where(tiny, 1e18, tmp)
    L = tmp / tmp.sum(-1, keepdims=True)                   # [B, NL, MN]

    C = w[:, :, None] * L                                  # [B, NL, MN]
    cmat = np.ascontiguousarray(
        C.transpose(1, 2, 0).reshape(NL, MN * B)
    ).astype(np.float16)

    node_scale = np.broadcast_to((2.0 * nodes)[None, :], (NL, MN))
    node_bias = np.broadcast_to((-(nodes ** 2))[None, :], (NL, MN))
    return {
        "node_scale": np.ascontiguousarray(node_scale, dtype=np.float32),
        "node_bias": np.ascontiguousarray(node_bias, dtype=np.float32),
        "cmat": cmat,
        "wu_t": np.ascontiguousarray(u.T).astype(ml_dtypes.bfloat16),
        "wv_t": np.ascontiguousarray(v.T).astype(ml_dtypes.bfloat16),
        "s_t": s.astype(np.float32).reshape(NL, 1),
    }


def shard_entities(E, numerical_literals):
    """Per-core [NE_PAD, D]/[NE_PAD, NL] slices, zero-padded."""
    E = np.asarray(E, dtype=np.float32)
    lit = np.asarray(numerical_literals, dtype=np.float32)
    e_slices, lit_slices, spans = [], [], []
    for core in range(NCORES):
        lo = core * NE_CORE
        hi = min(NE, lo + NE_CORE)
        es = np.zeros((NE_PAD, D), dtype=ml_dtypes.bfloat16)
        ls = np.zeros((NE_PAD, NL), dtype=np.float32)
        es[: hi - lo] = E[lo:hi].astype(ml_dtypes.bfloat16)
        ls[: hi - lo] = lit[lo:hi]
        e_slices.append(es)
        lit_slices.append(ls)
        spans.append((lo, hi))
    return e_slices, lit_slices, spans


def _make_in_maps(inputs):
    small = host_prep(**inputs)
    e_slices, lit_slices, spans = shard_entities(
        inputs["E"], inputs["numerical_literals"]
    )
    in_maps = []
    for core in range(NCORES):
        m = dict(small)
        m["e_slice"] = e_slices[core]
        m["lit_slice"] = lit_slices[core]
        in_maps.append(m)
    return in_maps, spans


def run_on_device(inputs, trace=False):
    nc = _get_nc()
    in_maps, spans = _make_in_maps(inputs)
    res = run_bass_kernel_spmd(nc, in_maps, core_ids=list(range(NCORES)), trace=trace)
    out = np.empty((B, NE), dtype=np.float32)
    for core, (lo, hi) in enumerate(spans):
        out[:, lo:hi] = res.results[core]["out"][:, : hi - lo]
    return out, res


def kernel(**inputs):
    out, _ = run_on_device(inputs, trace=False)
    return out


def _make_runner(nc, in_maps):
    """Build a reusable jitted callable + device-resident args for `nc`."""
    import jax
    from jax.sharding import Mesh, PartitionSpec
    try:
        from jax.experimental.shard_map import shard_map
    except ImportError:
        from jax.shard_map import shard_map
    from concourse import bass2jax

    bass2jax.install_neuronx_cc_hook()
    partition_name = nc.partition_id_tensor.name if nc.partition_id_tensor else None
    in_names, out_names, out_avals, zero_outs = [], [], [], []
    for alloc in nc.m.functions[0].allocations:
        if not isinstance(alloc, mybir.MemoryLocationSet):
            continue
        name = alloc.memorylocations[0].name
        if alloc.kind == "ExternalInput":
            if name != partition_name:
                in_names.append(name)
        elif alloc.kind == "ExternalOutput":
            shape = tuple(alloc.tensor_shape)
            dtype = mybir.dt.np(alloc.dtype)
            out_avals.append(jax.core.ShapedArray(shape, dtype))
            out_names.append(name)
            zero_outs.append(np.zeros(shape, dtype))
    n_params = len(in_names)
    all_names = list(in_names) + list(out_names)
    if partition_name is not None:
        all_names.append(partition_name)

    def _body(*args):
        operands = list(args)
        if partition_name is not None:
            operands.append(bass2jax.partition_id_tensor())
        return tuple(bass2jax._bass_exec_p.bind(
            *operands,
            out_avals=tuple(out_avals),
            in_names=tuple(all_names),
            out_names=tuple(out_names),
            lowering_input_output_aliases=(),
            sim_require_finite=True,
            sim_require_nnan=True,
            nc=nc,
        ))

    devices = jax.devices()[:NCORES]
    mesh = Mesh(np.asarray(devices), ("core",))
    nin = n_params + len(out_avals)
    per_core = [[np.asarray(m[nm]) for nm in in_names] for m in in_maps]
    concat_in = [np.concatenate([per_core[c][i] for c in range(NCORES)], axis=0)
                 for i in range(n_params)]
    concat_zeros = [np.zeros((NCORES * z.shape[0], *z.shape[1:]), z.dtype)
                    for z in zero_outs]
    f = jax.jit(shard_map(
        _body, mesh=mesh,
        in_specs=(PartitionSpec("core"),) * nin,
        out_specs=(PartitionSpec("core"),) * len(out_names),
        check_rep=False))
    args_dev = jax.device_put(
        concat_in + concat_zeros,
        [jax.sharding.NamedSharding(mesh, PartitionSpec("core"))] * nin)
    return f, args_dev


def bench(inputs, reps_program=64, timing_reps=100):
    """Per-execution device time: difference a program with the kernel body
    instantiated `reps_program` times against the 1-rep program. The (large,
    ~90 ms) axon dispatch overhead cancels in the difference."""
    import jax
    import time

    in_maps, _ = _make_in_maps(inputs)

    def timeit(f, args, n):
        jax.block_until_ready(f(*args))
        best = float("inf")
        for _ in range(n):
            t0 = time.perf_counter()
            jax.block_until_ready(f(*args))
            best = min(best, time.perf_counter() - t0)
        return best

    f1, a1 = _make_runner(_get_nc(1), in_maps)
    fR, aR = _make_runner(_get_nc(reps_program), in_maps)
    # warm both (compile + first dispatch)
    jax.block_until_ready(f1(*a1))
    jax.block_until_ready(fR(*aR))
    # interleave to cancel axon dispatch-time drift
    diffs = []
    for _ in range(timing_reps):
        t0 = time.perf_counter()
        jax.block_until_ready(f1(*a1))
        t1 = time.perf_counter()
        jax.block_until_ready(fR(*aR))
        t2 = time.perf_counter()
        diffs.append((t2 - t1) - (t1 - t0))
    diffs.sort()
    med = diffs[len(diffs) // 2]
    per = med / (reps_program - 1)
    print(f"bench: median extra for {reps_program - 1} reps = {med*1e3:.3f} ms"
          f"  -> per-exec {per*1e6:.1f} us"
          f"  (p25 {diffs[len(diffs)//4]/(reps_program-1)*1e6:.1f},"
          f" p75 {diffs[3*len(diffs)//4]/(reps_program-1)*1e6:.1f})")
    return per * 1e9
